# revision 13
# baseline (speedup 1.0000x reference)
"""Trainium2 Bass kernel for nn_CompressedMoE_31550829757014.

The reference's router/top-k computation is dead code -- the output is just
    out = x @ expert_w[0].T + expert_b[0]
i.e. one (8192 x 2048) x (2048 x 2048) GEMM with a bias.

Strategy:
  * Data-parallel over tokens: 8192 tokens / 8 cores = 1024 tokens per core.
  * Host-side prep: transpose x-shard and W0 so the contraction dim (d) lands
    on SBUF partitions; cast both operands to fp16.
  * Single-pass fp16 GEMM (fp32 PSUM accumulation): 1.0 PE cycles/row --
    the compute floor for this tolerance (harness gate 2e-2; fp16 gives
    ~2e-4).  fp8e4 DoubleRow (0.5 cyc/row) would land ~4e-2: over the gate.
    The split3 mode (3-term bf16 hi/lo GEMM, ~4e-6) is kept as fallback.
  * Device: W0T resident in SBUF (8 MB), x streamed per 128-token
    tile, dense back-to-back matmuls (PE stays warm), bias fused into the
    PSUM->SBUF copyback on the vector engine.
"""

import numpy as np
import ml_dtypes

BF16 = ml_dtypes.bfloat16

B, S, D, E = 4, 2048, 2048, 8
N_CORES = 8
T_CORE = (B * S) // N_CORES  # 1024 tokens per core


def _build_nc_single(T, DD, O, n_tile=512, mm_dtype_name="bfloat16"):
    """Single-pass GEMM: out[T,O] = xh.T @ wh + bias, one dtype, one pass.

    Schedule notes (all timings at 2.4 GHz PE, ~358 GB/s DMA):
      * 512-row bf16 matmul issues every ~223 ns; per-core PE floor 114 us.
      * W (8 MB) streams k-major on the sync queue; x m-tiles (512 KB) and
        out stores go on the scalar queue (second HWDGE queue) so trigger
        issue (~0.65 us each) doesn't serialize against the W stream.
      * m=0 and m=1 k-loops are interleaved: 8 matmuls (~1.7 us) per wh[k]
        arrival (~1.4 us) keeps the PE compute-paced, not DMA-paced, while
        W streams in.  Uses all 8 PSUM banks.
      * First matmul only needs xh[0][k=0] (32 KB) + wh[0][:, :512]
        (128 KB): both are split out as separate small tiles so the PE
        starts ~4 us earlier than waiting for the full 1 MB pair.
      * Last m-tile runs bank-at-a-time so the final copyback + store
        overlaps the remaining banks' matmuls.
    """
    import concourse.bacc as bacc
    import concourse.mybir as mybir
    import concourse.tile as tile
    from concourse.bass import ts

    P = 128
    KT = DD // P          # contraction tiles
    MT = T // P           # token tiles
    NT = O // n_tile      # output-feature tiles

    nc = bacc.Bacc(
        "TRN2", target_bir_lowering=False, debug=False, num_devices=N_CORES
    )
    f32 = mybir.dt.float32
    mdt = getattr(mybir.dt, mm_dtype_name)

    xh = nc.declare_dram_parameter("xh", [MT, P, KT, P], mdt, isOutput=False)
    wh = nc.declare_dram_parameter("wh", [DD, O], mdt, isOutput=False)
    bias = nc.declare_dram_parameter("bias", [1, O], f32, isOutput=False)
    out = nc.declare_dram_parameter("out", [T, O], f32, isOutput=True)
    wh_r = wh.rearrange("(k p) o -> p k o", p=P)

    with tile.TileContext(nc) as tc:
        with (
            tc.tile_pool(name="wpool", bufs=1) as wpool,
            tc.tile_pool(name="xpool", bufs=5) as xpool,
            tc.tile_pool(name="opool", bufs=8) as opool,
            tc.tile_pool(name="psum", bufs=8, space="PSUM") as psum,
        ):
            x_tiles = {}

            def load_x(m):
                t = xpool.tile([P, KT, P], mdt, tag="xh", name=f"xh_{m}")
                nc.scalar.dma_start(t[:], xh[m])
                x_tiles[m] = t

            # --- DMA issue order ---
            # scalar queue: x0 (k=0 chunk, then rest), x1, later prefetches +
            # out stores.  sync queue: wh[0] (n=0 chunk, then rest),
            # wh[1..KT-1], bias.
            x0_k0 = xpool.tile([P, 1, P], mdt, tag="x0k0")
            nc.scalar.dma_start(x0_k0[:], xh[0][:, 0:1])
            w0_n0 = wpool.tile([P, n_tile], mdt, tag="w0n0")
            nc.sync.dma_start(w0_n0[:], wh_r[:, 0, 0:n_tile])
            x0_rest = xpool.tile([P, KT - 1, P], mdt, tag="x0r")
            nc.scalar.dma_start(x0_rest[:], xh[0][:, 1:KT])
            w0_rest = wpool.tile([P, O - n_tile], mdt, tag="w0r")
            nc.sync.dma_start(w0_rest[:], wh_r[:, 0, n_tile:O])
            load_x(1)

            # Bias: fetch one 8 KB row, broadcast on the idle Pool engine --
            # keeps 1 MB off the DMA critical window while W streams.
            bias_sm = wpool.tile([1, O], f32, tag="bias_sm")
            nc.scalar.dma_start(bias_sm[:], bias[:])
            bias_sb = wpool.tile([P, O], f32, tag="bias")
            nc.gpsimd.partition_broadcast(bias_sb[:], bias_sm[0:1])

            wh_sb = [None] * KT
            for k in range(1, KT):
                t = wpool.tile([P, O], mdt, tag=f"wh{k}", name=f"wh_sb{k}")
                nc.sync.dma_start(t[:], wh_r[:, k])
                wh_sb[k] = t

            def x_ap(m, k):
                if m == 0:
                    return x0_k0[:, 0] if k == 0 else x0_rest[:, k - 1]
                return x_tiles[m][:, k]

            def w_ap(k, n):
                if k == 0:
                    if n == 0:
                        return w0_n0[:]
                    return w0_rest[:, ts(n - 1, n_tile)]
                return wh_sb[k][:, ts(n, n_tile)]

            def emit_copyback(ps, m, n):
                ob = opool.tile([P, n_tile], f32, tag="ob", name=f"ob_{m}_{n}")
                nc.vector.tensor_add(
                    out=ob[:], in0=ps[n][:], in1=bias_sb[:, ts(n, n_tile)]
                )
                nc.scalar.dma_start(out[ts(m, P), ts(n, n_tile)], ob[:])

            def new_psums(m):
                return [
                    psum.tile([P, n_tile], f32, tag="ps", name=f"ps_{m}_{n}")
                    for n in range(NT)
                ]

            # --- m=0 + m=1 interleaved k-loop (W-stream paced) ---
            ps0 = new_psums(0)
            ps1 = new_psums(1)
            for k in range(KT):
                first, last = k == 0, k == KT - 1
                for n in range(NT):
                    nc.tensor.matmul(
                        ps0[n][:], x_ap(0, k), w_ap(k, n), start=first, stop=last
                    )
                if last:
                    for n in range(NT):
                        emit_copyback(ps0, 0, n)
                for n in range(NT):
                    nc.tensor.matmul(
                        ps1[n][:], x_ap(1, k), w_ap(k, n), start=first, stop=last
                    )
                if last:
                    for n in range(NT):
                        emit_copyback(ps1, 1, n)
                if k == 10:
                    load_x(2)
                if k == 14:
                    load_x(3)

            # --- m=2..MT-1 ---
            for m in range(2, MT):
                xt_check = x_tiles[m]  # loaded by prefetch
                ps = new_psums(m)
                if m == MT - 1:
                    # k-major through k=KT-2 (one LDWEIGHTS per k, not per
                    # (n,k)), then the final k-round bank-at-a-time so each
                    # bank's copyback starts as soon as its last matmul ends.
                    for k in range(KT - 1):
                        for n in range(NT):
                            nc.tensor.matmul(
                                ps[n][:], x_ap(m, k), w_ap(k, n),
                                start=(k == 0), stop=False,
                            )
                    for n in range(NT):
                        nc.tensor.matmul(
                            ps[n][:], x_ap(m, KT - 1), w_ap(KT - 1, n),
                            start=False, stop=True,
                        )
                        emit_copyback(ps, m, n)
                else:
                    for k in range(KT):
                        for n in range(NT):
                            nc.tensor.matmul(
                                ps[n][:], x_ap(m, k), w_ap(k, n),
                                start=(k == 0), stop=(k == KT - 1),
                            )
                    for n in range(NT):
                        emit_copyback(ps, m, n)
                x_tiles.pop(m)
                if m + 2 < MT:
                    load_x(m + 2)

    nc.compile()
    return nc


def _build_nc(T, DD, O, n_tile=512, mode="single", mm_dtype_name="bfloat16"):
    if mode == "single":
        return _build_nc_single(T, DD, O, n_tile, mm_dtype_name)
    """Build the per-core Bass program: out[T,O] = xT.T @ w0T + bias.

    DRAM params (per core):
      xh, xl : [T/128, 128, DD/128, 128]  x-shard transposed + pre-tiled
               ([m,p,k,t] = xT[k*128+p, m*128+t]), hi/lo split
      wh, wl : [DD, O]  W0.T, hi/lo split (replicated across cores)
      bias   : [128, O] f32  b0 broadcast to 128 partitions (replicated)
      out    : [T, O]   f32

    mode="split3": psum += xh@wh + xh@wl + xl@wh (3-term split GEMM,
                   rel err ~4e-6 vs fp32)
    mode="single": psum += xh@wh only (xl/wl absent; used to probe dtypes --
                   float32r measured 156 us but rel err 1.2e-4)
    """
    import concourse.bacc as bacc
    import concourse.mybir as mybir
    import concourse.tile as tile
    from concourse.bass import ts

    P = 128
    KT = DD // P          # k tiles
    MT = T // P           # token tiles
    NT = O // n_tile      # output-feature tiles

    nc = bacc.Bacc(
        "TRN2", target_bir_lowering=False, debug=False, num_devices=N_CORES
    )
    f32 = mybir.dt.float32
    bf16 = getattr(mybir.dt, mm_dtype_name)

    # x is fed pre-tiled: [MT, P, KT, P] with [m, p, k, t] = xT[k*P+p, m*P+t],
    # so each m-tile's load is 128 partitions x 4KB contiguous.
    single = mode == "single"
    xh = nc.declare_dram_parameter("xh", [MT, P, KT, P], bf16, isOutput=False)
    wh = nc.declare_dram_parameter("wh", [DD, O], bf16, isOutput=False)
    if not single:
        xl = nc.declare_dram_parameter("xl", [MT, P, KT, P], bf16, isOutput=False)
        wl = nc.declare_dram_parameter("wl", [DD, O], bf16, isOutput=False)
        wl_r = wl.rearrange("(k p) o -> p k o", p=P)
    bias = nc.declare_dram_parameter("bias", [P, O], f32, isOutput=False)
    out = nc.declare_dram_parameter("out", [T, O], f32, isOutput=True)

    wh_r = wh.rearrange("(k p) o -> p k o", p=P)

    with tile.TileContext(nc) as tc:
        with (
            tc.tile_pool(name="wpool", bufs=1) as wpool,
            tc.tile_pool(name="xpool", bufs=4) as xpool,
            tc.tile_pool(name="opool", bufs=8) as opool,
            tc.tile_pool(name="psum", bufs=8, space="PSUM") as psum,
        ):
            x_tiles = {}

            def load_xh(m):
                xh_t = xpool.tile([P, KT, P], bf16, tag="xh", name=f"xh_{m}")
                nc.sync.dma_start(xh_t[:], xh[m])
                x_tiles[m] = (xh_t, None)

            def load_xl(m):
                if single:
                    return
                xl_t = xpool.tile([P, KT, P], bf16, tag="xl", name=f"xl_{m}")
                nc.sync.dma_start(xl_t[:], xl[m])
                x_tiles[m] = (x_tiles[m][0], xl_t)

            def load_x(m):
                load_xh(m)
                load_xl(m)

            # Resident weights, one tile per k-slice so matmuls only wait on
            # the k-slices they actually read. Emission order = DMA issue
            # order: the first matmul needs only xh[0] + wh k=0, so those go
            # first; the rest of W streams in k (use) order underneath the
            # compute.
            wh_sb = [None] * KT
            wl_sb = [None] * KT

            def load_wh(k):
                th = wpool.tile([P, O], bf16, tag=f"wh{k}", name=f"wh_sb{k}")
                nc.sync.dma_start(th[:], wh_r[:, k])
                wh_sb[k] = th

            def load_wl(k):
                if single:
                    return
                tl = wpool.tile([P, O], bf16, tag=f"wl{k}", name=f"wl_sb{k}")
                nc.sync.dma_start(tl[:], wl_r[:, k])
                wl_sb[k] = tl

            def load_bias():
                b = wpool.tile([P, O], f32, tag="bias")
                nc.sync.dma_start(b[:], bias[:])
                return b

            # Emission order tunes DMA issue order: the first matmul needs
            # only xh[m=0] + wh k=0, so exactly those go first. When m=0's
            # wl pass is deferred (defer_w), the whole wh stream goes before
            # any wl so m=0 is paced by wh arrival alone.
            defer_w = mode != "single" and MT >= 3
            bias_sb = None
            load_xh(0)
            load_wh(0)
            load_xl(0)
            if not defer_w:
                load_wl(0)
            for k in range(1, KT):
                load_wh(k)
                if not defer_w:
                    load_wl(k)
                if k == KT // 2 and MT > 1:
                    load_x(1)
                if k == (KT * 5) // 8 and not defer_w:
                    bias_sb = load_bias()
            if defer_w:
                for k in range(KT):
                    load_wl(k)
                    if k == (KT * 5) // 8:
                        bias_sb = load_bias()
            if MT > 1 and KT < 2:
                load_x(1)
            if bias_sb is None:
                bias_sb = load_bias()

            # m=0's wl-dependent pass (hl) is deferred into m=1's window:
            # while m=0 runs, the DMA stream only has to deliver wh
            # (~290 GB/s demand < ~325 GB/s supply), so m=0 is no longer
            # DMA-paced; the deferred matmuls run interleaved with m=1's
            # k-loop once the wl slices have arrived.
            defer_m0 = mode != "single" and MT >= 3
            psums0 = None
            xh0_sb = None

            for m in range(MT):
                xh_sb, xl_sb = x_tiles.pop(m)

                psums = [
                    psum.tile([P, n_tile], f32, tag="ps", name=f"ps_{m}_{n}")
                    for n in range(NT)
                ]
                def emit_mms(k, ns):
                    first = k == 0
                    last = k == KT - 1
                    # Pass order hh, lh, hl: both wh passes run before the wl
                    # pass so wl[k]'s DMA gets 8 matmuls of extra slack while
                    # m=0 is still DMA-paced.
                    for n in ns:
                        nc.tensor.matmul(
                            psums[n][:], xh_sb[:, k], wh_sb[k][:, ts(n, n_tile)],
                            start=first, stop=(last and mode == "single"),
                        )
                    if mode == "single":
                        return
                    for n in ns:
                        nc.tensor.matmul(
                            psums[n][:], xl_sb[:, k], wh_sb[k][:, ts(n, n_tile)],
                            start=False, stop=False,
                        )
                    for n in ns:
                        nc.tensor.matmul(
                            psums[n][:], xh_sb[:, k], wl_sb[k][:, ts(n, n_tile)],
                            start=False, stop=last,
                        )

                def emit_copyback(n, ps=None, mi=None):
                    ps = psums if ps is None else ps
                    mi = m if mi is None else mi
                    ob = opool.tile([P, n_tile], f32, tag="ob", name=f"ob_{mi}_{n}")
                    nc.vector.tensor_add(
                        out=ob[:], in0=ps[n][:], in1=bias_sb[:, ts(n, n_tile)]
                    )
                    nc.sync.dma_start(out[ts(mi, P), ts(n, n_tile)], ob[:])

                if defer_m0 and m == 0:
                    # hh + lh passes only (wh-dependent); hl is deferred.
                    for k in range(KT):
                        for n in range(NT):
                            nc.tensor.matmul(
                                psums[n][:], xh_sb[:, k],
                                wh_sb[k][:, ts(n, n_tile)],
                                start=(k == 0), stop=False,
                            )
                        for n in range(NT):
                            nc.tensor.matmul(
                                psums[n][:], xl_sb[:, k],
                                wh_sb[k][:, ts(n, n_tile)],
                                start=False, stop=False,
                            )
                    psums0 = psums
                    xh0_sb = xh_sb
                elif defer_m0 and m == 1:
                    for k in range(KT):
                        emit_mms(k, list(range(NT)))
                        # m=0's deferred hl pass, one k-slice per m=1 k-step
                        for n in range(NT):
                            nc.tensor.matmul(
                                psums0[n][:], xh0_sb[:, k],
                                wl_sb[k][:, ts(n, n_tile)],
                                start=False, stop=(k == KT - 1),
                            )
                    for n in range(NT):
                        emit_copyback(n, ps=psums0, mi=0)
                    for n in range(NT):
                        emit_copyback(n)
                elif m == MT - 1:
                    # Last m-tile: finish one psum bank at a time so the
                    # copyback + store of bank n overlaps bank n+1's matmuls
                    # instead of all serializing after the final matmul.
                    for n in range(NT):
                        for k in range(KT):
                            emit_mms(k, [n])
                        emit_copyback(n)
                else:
                    for k in range(KT):
                        emit_mms(k, list(range(NT)))
                    for n in range(NT):
                        emit_copyback(n)

                if m + 2 < MT:
                    load_x(m + 2)

    nc.compile()
    return nc


def _split_bf16(a_f32):
    """Split fp32 array into bf16 hi + bf16 lo with x ~= hi + lo."""
    hi = a_f32.astype(BF16)
    lo = (a_f32 - hi.astype(np.float32)).astype(BF16)
    return hi, lo


def _tile_xT(xt_2d):
    """[D, T] -> [T//128, 128, D//128, 128] with [m,p,k,t] = xt[k*128+p, m*128+t]."""
    DD, T = xt_2d.shape
    return np.ascontiguousarray(
        xt_2d.reshape(DD // 128, 128, T // 128, 128).transpose(2, 1, 0, 3)
    )


def _prep_in_maps(x, expert_w, expert_b, mode="single", mm_dtype=BF16):
    x2 = np.asarray(x, dtype=np.float32).reshape(B * S, D)
    w0t = np.ascontiguousarray(np.asarray(expert_w, dtype=np.float32)[0].T)  # [D, O]
    single = mode == "single"
    bias_rows = 1 if single else 128
    bias = np.ascontiguousarray(
        np.broadcast_to(
            np.asarray(expert_b, dtype=np.float32)[0], (bias_rows, D)
        ).astype(np.float32)
    )
    if single:
        wh = w0t.astype(mm_dtype)
    else:
        wh, wl = _split_bf16(w0t)
    in_maps = []
    for c in range(N_CORES):
        xct = x2[c * T_CORE : (c + 1) * T_CORE].T  # [D, T] view
        if single:
            m = {"xh": _tile_xT(xct.astype(mm_dtype)), "wh": wh, "bias": bias}
        else:
            xh, xl = _split_bf16(xct)
            m = {
                "xh": _tile_xT(xh),
                "xl": _tile_xT(xl),
                "wh": wh,
                "wl": wl,
                "bias": bias,
            }
        in_maps.append(m)
    return in_maps


_NC_CACHE = {}


def kernel(x, router_w, expert_w, expert_b):
    from concourse.bass_utils import run_bass_kernel_spmd

    in_maps = _prep_in_maps(x, expert_w, expert_b)
    if "nc" not in _NC_CACHE:
        _NC_CACHE["nc"] = _build_nc(T_CORE, D, D)
    nc = _NC_CACHE["nc"]
    res = run_bass_kernel_spmd(nc, in_maps, list(range(N_CORES)))
    outs = [res.results[c]["out"] for c in range(N_CORES)]
    full = np.concatenate(outs, axis=0).reshape(B, S, D)
    return np.ascontiguousarray(full.astype(np.float32))



# revision 15
# speedup vs baseline: 1.0284x; 1.0284x over previous
"""Trainium2 Bass kernel for nn_CompressedMoE_31550829757014.

The reference's router/top-k computation is dead code -- the output is just
    out = x @ expert_w[0].T + expert_b[0]
i.e. one (8192 x 2048) x (2048 x 2048) GEMM with a bias.

Strategy:
  * Data-parallel over tokens: 8192 tokens / 8 cores = 1024 tokens per core.
  * Host-side prep: transpose x-shard and W0 so the contraction dim (d) lands
    on SBUF partitions; cast both operands to fp16.
  * Single-pass fp16 GEMM (fp32 PSUM accumulation): 1.0 PE cycles/row --
    the compute floor for this tolerance (harness gate 2e-2; fp16 gives
    ~2e-4).  fp8e4 DoubleRow (0.5 cyc/row) would land ~4e-2: over the gate.
    The split3 mode (3-term bf16 hi/lo GEMM, ~4e-6) is kept as fallback.
  * Device: W0T resident in SBUF (8 MB), x streamed per 128-token
    tile, dense back-to-back matmuls (PE stays warm), bias fused into the
    PSUM->SBUF copyback on the vector engine.
"""

import numpy as np
import ml_dtypes

BF16 = ml_dtypes.bfloat16

B, S, D, E = 4, 2048, 2048, 8
N_CORES = 8
T_CORE = (B * S) // N_CORES  # 1024 tokens per core


def _build_nc_single(T, DD, O, n_tile=512, mm_dtype_name="bfloat16"):
    """Single-pass GEMM: out[T,O] = xh.T @ wh + bias, one dtype, one pass.

    Schedule notes (all timings at 2.4 GHz PE, ~358 GB/s DMA):
      * 512-row bf16 matmul issues every ~223 ns; per-core PE floor 114 us.
      * W (8 MB) streams k-major on the sync queue; x m-tiles (512 KB) and
        out stores go on the scalar queue (second HWDGE queue) so trigger
        issue (~0.65 us each) doesn't serialize against the W stream.
      * m=0 and m=1 k-loops are interleaved: 8 matmuls (~1.7 us) per wh[k]
        arrival (~1.4 us) keeps the PE compute-paced, not DMA-paced, while
        W streams in.  Uses all 8 PSUM banks.
      * First matmul only needs xh[0][k=0] (32 KB) + wh[0][:, :512]
        (128 KB): both are split out as separate small tiles so the PE
        starts ~4 us earlier than waiting for the full 1 MB pair.
      * Last m-tile runs bank-at-a-time so the final copyback + store
        overlaps the remaining banks' matmuls.
    """
    import concourse.bacc as bacc
    import concourse.mybir as mybir
    import concourse.tile as tile
    from concourse.bass import ts

    P = 128
    KT = DD // P          # contraction tiles
    MT = T // P           # token tiles
    NT = O // n_tile      # output-feature tiles

    nc = bacc.Bacc(
        "TRN2", target_bir_lowering=False, debug=False, num_devices=N_CORES
    )
    f32 = mybir.dt.float32
    mdt = getattr(mybir.dt, mm_dtype_name)

    xh = nc.declare_dram_parameter("xh", [MT, P, KT, P], mdt, isOutput=False)
    wh = nc.declare_dram_parameter("wh", [DD, O], mdt, isOutput=False)
    bias = nc.declare_dram_parameter("bias", [1, O], f32, isOutput=False)
    out = nc.declare_dram_parameter("out", [T, O], f32, isOutput=True)
    wh_r = wh.rearrange("(k p) o -> p k o", p=P)

    with tile.TileContext(nc) as tc:
        with (
            tc.tile_pool(name="wpool", bufs=1) as wpool,
            tc.tile_pool(name="xpool", bufs=5) as xpool,
            tc.tile_pool(name="opool", bufs=8) as opool,
            tc.tile_pool(name="psum", bufs=8, space="PSUM") as psum,
        ):
            x_tiles = {}

            def load_x(m):
                t = xpool.tile([P, KT, P], mdt, tag="xh", name=f"xh_{m}")
                nc.scalar.dma_start(t[:], xh[m])
                x_tiles[m] = t

            # --- DMA issue order ---
            # scalar queue: x0 (k=0 chunk, then rest), x1, later prefetches +
            # out stores.  sync queue: wh[0] (n=0 chunk, then rest),
            # wh[1..KT-1], bias.
            x0_k0 = xpool.tile([P, 1, P], mdt, tag="x0k0")
            nc.scalar.dma_start(x0_k0[:], xh[0][:, 0:1])
            w0_n0 = wpool.tile([P, n_tile], mdt, tag="w0n0")
            nc.sync.dma_start(w0_n0[:], wh_r[:, 0, 0:n_tile])
            x0_rest = xpool.tile([P, KT - 1, P], mdt, tag="x0r")
            nc.scalar.dma_start(x0_rest[:], xh[0][:, 1:KT])
            w0_rest = wpool.tile([P, O - n_tile], mdt, tag="w0r")
            nc.sync.dma_start(w0_rest[:], wh_r[:, 0, n_tile:O])
            load_x(1)

            # Bias: fetch one 8 KB row, broadcast on the idle Pool engine --
            # keeps 1 MB off the DMA critical window while W streams.
            bias_sm = wpool.tile([1, O], f32, tag="bias_sm")
            nc.scalar.dma_start(bias_sm[:], bias[:])
            bias_sb = wpool.tile([P, O], f32, tag="bias")
            nc.gpsimd.partition_broadcast(bias_sb[:], bias_sm[0:1])

            wh_sb = [None] * KT
            for k in range(1, KT):
                t = wpool.tile([P, O], mdt, tag=f"wh{k}", name=f"wh_sb{k}")
                nc.sync.dma_start(t[:], wh_r[:, k])
                wh_sb[k] = t

            def x_ap(m, k):
                if m == 0:
                    return x0_k0[:, 0] if k == 0 else x0_rest[:, k - 1]
                return x_tiles[m][:, k]

            def w_ap(k, n):
                if k == 0:
                    if n == 0:
                        return w0_n0[:]
                    return w0_rest[:, ts(n - 1, n_tile)]
                return wh_sb[k][:, ts(n, n_tile)]

            def emit_copyback(ps, m, n):
                ob = opool.tile([P, n_tile], f32, tag="ob", name=f"ob_{m}_{n}")
                nc.vector.tensor_add(
                    out=ob[:], in0=ps[n][:], in1=bias_sb[:, ts(n, n_tile)]
                )
                # stores go on the sync queue: it is idle once W has streamed
                # in, while scalar still carries x prefetches
                nc.sync.dma_start(out[ts(m, P), ts(n, n_tile)], ob[:])

            def new_psums(m):
                return [
                    psum.tile([P, n_tile], f32, tag="ps", name=f"ps_{m}_{n}")
                    for n in range(NT)
                ]

            # --- m=0 + m=1 interleaved k-loop (W-stream paced) ---
            ps0 = new_psums(0)
            ps1 = new_psums(1)
            for k in range(KT):
                first, last = k == 0, k == KT - 1
                for n in range(NT):
                    nc.tensor.matmul(
                        ps0[n][:], x_ap(0, k), w_ap(k, n), start=first, stop=last
                    )
                if last:
                    for n in range(NT):
                        emit_copyback(ps0, 0, n)
                for n in range(NT):
                    nc.tensor.matmul(
                        ps1[n][:], x_ap(1, k), w_ap(k, n), start=first, stop=last
                    )
                if last:
                    for n in range(NT):
                        emit_copyback(ps1, 1, n)
                if k == 10:
                    load_x(2)
                if k == 14:
                    load_x(3)

            # --- m=2..MT-1 ---
            for m in range(2, MT):
                xt_check = x_tiles[m]  # loaded by prefetch
                ps = new_psums(m)
                if m == MT - 1:
                    # k-major for most of the contraction (one LDWEIGHTS per
                    # k), then stagger the bank finishes: bank n runs its last
                    # KF k-steps alone and fires its copyback immediately, so
                    # copyback+store of bank n overlap bank n+1's final
                    # matmuls instead of all piling up after the last one.
                    KF = 4
                    for k in range(KT - KF):
                        for n in range(NT):
                            nc.tensor.matmul(
                                ps[n][:], x_ap(m, k), w_ap(k, n),
                                start=(k == 0), stop=False,
                            )
                    for n in range(NT):
                        for k in range(KT - KF, KT):
                            nc.tensor.matmul(
                                ps[n][:], x_ap(m, k), w_ap(k, n),
                                start=False, stop=(k == KT - 1),
                            )
                        emit_copyback(ps, m, n)
                else:
                    for k in range(KT):
                        for n in range(NT):
                            nc.tensor.matmul(
                                ps[n][:], x_ap(m, k), w_ap(k, n),
                                start=(k == 0), stop=(k == KT - 1),
                            )
                    for n in range(NT):
                        emit_copyback(ps, m, n)
                x_tiles.pop(m)
                if m + 2 < MT:
                    load_x(m + 2)

    nc.compile()
    return nc


def _build_nc(T, DD, O, n_tile=512, mode="single", mm_dtype_name="bfloat16"):
    if mode == "single":
        return _build_nc_single(T, DD, O, n_tile, mm_dtype_name)
    """Build the per-core Bass program: out[T,O] = xT.T @ w0T + bias.

    DRAM params (per core):
      xh, xl : [T/128, 128, DD/128, 128]  x-shard transposed + pre-tiled
               ([m,p,k,t] = xT[k*128+p, m*128+t]), hi/lo split
      wh, wl : [DD, O]  W0.T, hi/lo split (replicated across cores)
      bias   : [128, O] f32  b0 broadcast to 128 partitions (replicated)
      out    : [T, O]   f32

    mode="split3": psum += xh@wh + xh@wl + xl@wh (3-term split GEMM,
                   rel err ~4e-6 vs fp32)
    mode="single": psum += xh@wh only (xl/wl absent; used to probe dtypes --
                   float32r measured 156 us but rel err 1.2e-4)
    """
    import concourse.bacc as bacc
    import concourse.mybir as mybir
    import concourse.tile as tile
    from concourse.bass import ts

    P = 128
    KT = DD // P          # k tiles
    MT = T // P           # token tiles
    NT = O // n_tile      # output-feature tiles

    nc = bacc.Bacc(
        "TRN2", target_bir_lowering=False, debug=False, num_devices=N_CORES
    )
    f32 = mybir.dt.float32
    bf16 = getattr(mybir.dt, mm_dtype_name)

    # x is fed pre-tiled: [MT, P, KT, P] with [m, p, k, t] = xT[k*P+p, m*P+t],
    # so each m-tile's load is 128 partitions x 4KB contiguous.
    single = mode == "single"
    xh = nc.declare_dram_parameter("xh", [MT, P, KT, P], bf16, isOutput=False)
    wh = nc.declare_dram_parameter("wh", [DD, O], bf16, isOutput=False)
    if not single:
        xl = nc.declare_dram_parameter("xl", [MT, P, KT, P], bf16, isOutput=False)
        wl = nc.declare_dram_parameter("wl", [DD, O], bf16, isOutput=False)
        wl_r = wl.rearrange("(k p) o -> p k o", p=P)
    bias = nc.declare_dram_parameter("bias", [P, O], f32, isOutput=False)
    out = nc.declare_dram_parameter("out", [T, O], f32, isOutput=True)

    wh_r = wh.rearrange("(k p) o -> p k o", p=P)

    with tile.TileContext(nc) as tc:
        with (
            tc.tile_pool(name="wpool", bufs=1) as wpool,
            tc.tile_pool(name="xpool", bufs=4) as xpool,
            tc.tile_pool(name="opool", bufs=8) as opool,
            tc.tile_pool(name="psum", bufs=8, space="PSUM") as psum,
        ):
            x_tiles = {}

            def load_xh(m):
                xh_t = xpool.tile([P, KT, P], bf16, tag="xh", name=f"xh_{m}")
                nc.sync.dma_start(xh_t[:], xh[m])
                x_tiles[m] = (xh_t, None)

            def load_xl(m):
                if single:
                    return
                xl_t = xpool.tile([P, KT, P], bf16, tag="xl", name=f"xl_{m}")
                nc.sync.dma_start(xl_t[:], xl[m])
                x_tiles[m] = (x_tiles[m][0], xl_t)

            def load_x(m):
                load_xh(m)
                load_xl(m)

            # Resident weights, one tile per k-slice so matmuls only wait on
            # the k-slices they actually read. Emission order = DMA issue
            # order: the first matmul needs only xh[0] + wh k=0, so those go
            # first; the rest of W streams in k (use) order underneath the
            # compute.
            wh_sb = [None] * KT
            wl_sb = [None] * KT

            def load_wh(k):
                th = wpool.tile([P, O], bf16, tag=f"wh{k}", name=f"wh_sb{k}")
                nc.sync.dma_start(th[:], wh_r[:, k])
                wh_sb[k] = th

            def load_wl(k):
                if single:
                    return
                tl = wpool.tile([P, O], bf16, tag=f"wl{k}", name=f"wl_sb{k}")
                nc.sync.dma_start(tl[:], wl_r[:, k])
                wl_sb[k] = tl

            def load_bias():
                b = wpool.tile([P, O], f32, tag="bias")
                nc.sync.dma_start(b[:], bias[:])
                return b

            # Emission order tunes DMA issue order: the first matmul needs
            # only xh[m=0] + wh k=0, so exactly those go first. When m=0's
            # wl pass is deferred (defer_w), the whole wh stream goes before
            # any wl so m=0 is paced by wh arrival alone.
            defer_w = mode != "single" and MT >= 3
            bias_sb = None
            load_xh(0)
            load_wh(0)
            load_xl(0)
            if not defer_w:
                load_wl(0)
            for k in range(1, KT):
                load_wh(k)
                if not defer_w:
                    load_wl(k)
                if k == KT // 2 and MT > 1:
                    load_x(1)
                if k == (KT * 5) // 8 and not defer_w:
                    bias_sb = load_bias()
            if defer_w:
                for k in range(KT):
                    load_wl(k)
                    if k == (KT * 5) // 8:
                        bias_sb = load_bias()
            if MT > 1 and KT < 2:
                load_x(1)
            if bias_sb is None:
                bias_sb = load_bias()

            # m=0's wl-dependent pass (hl) is deferred into m=1's window:
            # while m=0 runs, the DMA stream only has to deliver wh
            # (~290 GB/s demand < ~325 GB/s supply), so m=0 is no longer
            # DMA-paced; the deferred matmuls run interleaved with m=1's
            # k-loop once the wl slices have arrived.
            defer_m0 = mode != "single" and MT >= 3
            psums0 = None
            xh0_sb = None

            for m in range(MT):
                xh_sb, xl_sb = x_tiles.pop(m)

                psums = [
                    psum.tile([P, n_tile], f32, tag="ps", name=f"ps_{m}_{n}")
                    for n in range(NT)
                ]
                def emit_mms(k, ns):
                    first = k == 0
                    last = k == KT - 1
                    # Pass order hh, lh, hl: both wh passes run before the wl
                    # pass so wl[k]'s DMA gets 8 matmuls of extra slack while
                    # m=0 is still DMA-paced.
                    for n in ns:
                        nc.tensor.matmul(
                            psums[n][:], xh_sb[:, k], wh_sb[k][:, ts(n, n_tile)],
                            start=first, stop=(last and mode == "single"),
                        )
                    if mode == "single":
                        return
                    for n in ns:
                        nc.tensor.matmul(
                            psums[n][:], xl_sb[:, k], wh_sb[k][:, ts(n, n_tile)],
                            start=False, stop=False,
                        )
                    for n in ns:
                        nc.tensor.matmul(
                            psums[n][:], xh_sb[:, k], wl_sb[k][:, ts(n, n_tile)],
                            start=False, stop=last,
                        )

                def emit_copyback(n, ps=None, mi=None):
                    ps = psums if ps is None else ps
                    mi = m if mi is None else mi
                    ob = opool.tile([P, n_tile], f32, tag="ob", name=f"ob_{mi}_{n}")
                    nc.vector.tensor_add(
                        out=ob[:], in0=ps[n][:], in1=bias_sb[:, ts(n, n_tile)]
                    )
                    nc.sync.dma_start(out[ts(mi, P), ts(n, n_tile)], ob[:])

                if defer_m0 and m == 0:
                    # hh + lh passes only (wh-dependent); hl is deferred.
                    for k in range(KT):
                        for n in range(NT):
                            nc.tensor.matmul(
                                psums[n][:], xh_sb[:, k],
                                wh_sb[k][:, ts(n, n_tile)],
                                start=(k == 0), stop=False,
                            )
                        for n in range(NT):
                            nc.tensor.matmul(
                                psums[n][:], xl_sb[:, k],
                                wh_sb[k][:, ts(n, n_tile)],
                                start=False, stop=False,
                            )
                    psums0 = psums
                    xh0_sb = xh_sb
                elif defer_m0 and m == 1:
                    for k in range(KT):
                        emit_mms(k, list(range(NT)))
                        # m=0's deferred hl pass, one k-slice per m=1 k-step
                        for n in range(NT):
                            nc.tensor.matmul(
                                psums0[n][:], xh0_sb[:, k],
                                wl_sb[k][:, ts(n, n_tile)],
                                start=False, stop=(k == KT - 1),
                            )
                    for n in range(NT):
                        emit_copyback(n, ps=psums0, mi=0)
                    for n in range(NT):
                        emit_copyback(n)
                elif m == MT - 1:
                    # Last m-tile: finish one psum bank at a time so the
                    # copyback + store of bank n overlaps bank n+1's matmuls
                    # instead of all serializing after the final matmul.
                    for n in range(NT):
                        for k in range(KT):
                            emit_mms(k, [n])
                        emit_copyback(n)
                else:
                    for k in range(KT):
                        emit_mms(k, list(range(NT)))
                    for n in range(NT):
                        emit_copyback(n)

                if m + 2 < MT:
                    load_x(m + 2)

    nc.compile()
    return nc


def _split_bf16(a_f32):
    """Split fp32 array into bf16 hi + bf16 lo with x ~= hi + lo."""
    hi = a_f32.astype(BF16)
    lo = (a_f32 - hi.astype(np.float32)).astype(BF16)
    return hi, lo


def _tile_xT(xt_2d):
    """[D, T] -> [T//128, 128, D//128, 128] with [m,p,k,t] = xt[k*128+p, m*128+t]."""
    DD, T = xt_2d.shape
    return np.ascontiguousarray(
        xt_2d.reshape(DD // 128, 128, T // 128, 128).transpose(2, 1, 0, 3)
    )


def _prep_in_maps(x, expert_w, expert_b, mode="single", mm_dtype=BF16):
    x2 = np.asarray(x, dtype=np.float32).reshape(B * S, D)
    w0t = np.ascontiguousarray(np.asarray(expert_w, dtype=np.float32)[0].T)  # [D, O]
    single = mode == "single"
    bias_rows = 1 if single else 128
    bias = np.ascontiguousarray(
        np.broadcast_to(
            np.asarray(expert_b, dtype=np.float32)[0], (bias_rows, D)
        ).astype(np.float32)
    )
    if single:
        wh = w0t.astype(mm_dtype)
    else:
        wh, wl = _split_bf16(w0t)
    in_maps = []
    for c in range(N_CORES):
        xct = x2[c * T_CORE : (c + 1) * T_CORE].T  # [D, T] view
        if single:
            m = {"xh": _tile_xT(xct.astype(mm_dtype)), "wh": wh, "bias": bias}
        else:
            xh, xl = _split_bf16(xct)
            m = {
                "xh": _tile_xT(xh),
                "xl": _tile_xT(xl),
                "wh": wh,
                "wl": wl,
                "bias": bias,
            }
        in_maps.append(m)
    return in_maps


_NC_CACHE = {}


def kernel(x, router_w, expert_w, expert_b):
    from concourse.bass_utils import run_bass_kernel_spmd

    in_maps = _prep_in_maps(x, expert_w, expert_b)
    if "nc" not in _NC_CACHE:
        _NC_CACHE["nc"] = _build_nc(T_CORE, D, D)
    nc = _NC_CACHE["nc"]
    res = run_bass_kernel_spmd(nc, in_maps, list(range(N_CORES)))
    outs = [res.results[c]["out"] for c in range(N_CORES)]
    full = np.concatenate(outs, axis=0).reshape(B, S, D)
    return np.ascontiguousarray(full.astype(np.float32))

